# revision 2
# baseline (speedup 1.0000x reference)
"""Trainium2 Bass kernel for LocalSelfAttentionHeadSum.

Reference computation (per sample b of B=32):
  x = x_window[b] reshaped (C=1024, THW=1764); x_item = cols 784:980 (center frame)
  q = Wq @ x_item + bq          (512, 196)
  k = Wk @ x + bk               (512, 1764)
  v = Wv @ x + bv               (512, 1764)
  alpha = softmax(q^T k, axis=-1)
  y = v @ alpha^T               (512, 196)
  out = Wo @ y + bo             (1024, 196)

Sharding: data-parallel over B across 8 cores (4 samples per core).

Kernel structure per core (see design notes inline):
  - Q-projection batched over the 4 samples (moving dim 784 -> full-rate tf32)
  - per sample, stream key-chunks [512, 512, 370, 370]:
      K-proj (weights stationary) -> scores S=(q^T k) in (query-part, key-free)
      layout -> exp on ScalarE with fused row-sum (softmax denominator; no max
      subtraction needed: |S| <~ 60 so exp stays in fp32 range) -> PE-transpose
      exp(S) -> AV matmul accumulated in PSUM across chunks.
      V-proj produces v^T directly (x stationary, Wv^T moving).
  - normalize by 1/Z, PE-transpose y^T -> y, add bv
  - output projection batched over samples, + bo, DMA out.

Precision: fp32r (tf32) matmuls at full PE rate; Wq/Wk (and optionally Wv/Wo)
applied in two hi/lo tf32 passes to keep the q/k projection error small, since
score errors are amplified ~20x by the |q||k|/|S| cancellation + exp.
"""

import os
import numpy as np

import concourse.bass as bass
import concourse.tile as tile
from concourse import bacc, mybir
from concourse.bass_utils import run_bass_kernel_spmd
from concourse.masks import make_identity

F32 = mybir.dt.float32
F32R = mybir.dt.float32r

# Problem shapes (hardcoded per contract)
B, C, T, H, W = 32, 1024, 9, 14, 14
CI = 512
HW = H * W              # 196
THW = T * HW            # 1764
NCORES = 8
BPC = B // NCORES       # 4 samples per core
CT = C // 128           # 8 C-tiles
MI = CI // 128          # 4 Ci-tiles
ITEM0 = (T // 2) * HW   # 784, center-frame column offset
QT = [(0, 128), (128, HW - 128)]          # query partition tiles
CHUNKS = [(0, 512), (512, 512), (1024, 370), (1394, 370)]
SAMP = BPC * HW         # 784 batched (sample, query) columns
NCH = [(0, 512), (512, SAMP - 512)]       # batched free-dim chunks


def _subtiles(n):
    out = []
    o = 0
    while o < n:
        out.append((o, min(128, n - o)))
        o += 128
    return out


# Pass counts (tf32 hi/lo splits). 2 = hi/lo (near-fp32 on that operand).
NPASS_Q = int(os.environ.get("NPASS_Q", "2"))
NPASS_K = int(os.environ.get("NPASS_K", "2"))
NPASS_V = int(os.environ.get("NPASS_V", "1"))
NPASS_O = int(os.environ.get("NPASS_O", "1"))


def build_kernel():
    nc = bacc.Bacc("TRN2", target_bir_lowering=False, debug=False)

    x_d = nc.dram_tensor("x", [BPC, C, THW], F32, kind="ExternalInput")
    wqt_d = [nc.dram_tensor(f"wqt{i}", [C, CI], F32, kind="ExternalInput")
             for i in range(NPASS_Q)]
    wkt_d = [nc.dram_tensor(f"wkt{i}", [C, CI], F32, kind="ExternalInput")
             for i in range(NPASS_K)]
    wvt_d = [nc.dram_tensor(f"wvt{i}", [C, CI], F32, kind="ExternalInput")
             for i in range(NPASS_V)]
    wot_d = [nc.dram_tensor(f"wot{i}", [CI, C], F32, kind="ExternalInput")
             for i in range(NPASS_O)]
    bq_d = nc.dram_tensor("bq", [MI, 128], F32, kind="ExternalInput")
    bk_d = nc.dram_tensor("bk", [MI, 128], F32, kind="ExternalInput")
    bv_d = nc.dram_tensor("bv", [MI, 128], F32, kind="ExternalInput")
    bo_d = nc.dram_tensor("bo", [CT, 128], F32, kind="ExternalInput")
    out_d = nc.dram_tensor("out", [BPC, C, HW], F32, kind="ExternalOutput")

    with tile.TileContext(nc) as tc:
        with (
            tc.tile_pool(name="const", bufs=1) as const_pool,
            tc.tile_pool(name="persist", bufs=1) as persist,
        ):
            ident = const_pool.tile([128, 128], F32)
            make_identity(nc, ident[:])

            # Weights: [128 part, C-tile (or Ci-tile), free]
            wqt = [persist.tile([128, CT, CI], F32R, tag=f"wqt{i}", name=f"wqt{i}")
                   for i in range(NPASS_Q)]
            wkt = [persist.tile([128, CT, CI], F32R, tag=f"wkt{i}", name=f"wkt{i}")
                   for i in range(NPASS_K)]
            wvt = [persist.tile([128, CT, CI], F32R, tag=f"wvt{i}", name=f"wvt{i}")
                   for i in range(NPASS_V)]
            wot = [persist.tile([128, MI, C], F32R, tag=f"wot{i}", name=f"wot{i}")
                   for i in range(NPASS_O)]
            for i in range(NPASS_Q):
                nc.sync.dma_start(
                    wqt[i][:], wqt_d[i][:].rearrange("(t p) e -> p t e", p=128).bitcast(F32R))
            for i in range(NPASS_K):
                nc.sync.dma_start(
                    wkt[i][:], wkt_d[i][:].rearrange("(t p) e -> p t e", p=128).bitcast(F32R))
            for i in range(NPASS_V):
                nc.sync.dma_start(
                    wvt[i][:], wvt_d[i][:].rearrange("(t p) e -> p t e", p=128).bitcast(F32R))
            for i in range(NPASS_O):
                nc.sync.dma_start(
                    wot[i][:], wot_d[i][:].rearrange("(m p) c -> p m c", p=128).bitcast(F32R))

            bq_sb = const_pool.tile([128, MI], F32)
            bk_sb = const_pool.tile([128, MI], F32)
            bv_sb = const_pool.tile([128, MI], F32)
            bo_sb = const_pool.tile([128, CT], F32)
            for m in range(MI):
                nc.sync.dma_start(bq_sb[:, m:m + 1], bq_d[m])
                nc.sync.dma_start(bk_sb[:, m:m + 1], bk_d[m])
                nc.sync.dma_start(bv_sb[:, m:m + 1], bv_d[m])
            for m in range(CT):
                nc.sync.dma_start(bo_sb[:, m:m + 1], bo_d[m])

            q_all = persist.tile([128, MI, SAMP], F32R)   # (Ci-part, Ci-tile, (s,q))
            y_all = persist.tile([128, MI, SAMP], F32R)

            # ---------- Phase A: batched Q projection ----------
            with (
                tc.tile_pool(name="phaseA", bufs=1) as pA,
                tc.tile_pool(name="psA", bufs=2, space="PSUM") as psA,
            ):
                x_items = pA.tile([128, CT, SAMP], F32R)
                for s in range(BPC):
                    nc.sync.dma_start(
                        x_items[:, :, s * HW:(s + 1) * HW],
                        x_d[s, :, ITEM0:ITEM0 + HW]
                        .rearrange("(t p) n -> p t n", p=128).bitcast(F32R))
                for m in range(MI):
                    for (n0, nsz) in NCH:
                        pq = psA.tile([128, 512], F32, tag="pq")
                        for t in range(CT):
                            for ip in range(NPASS_Q):
                                nc.tensor.matmul(
                                    pq[:, :nsz],
                                    wqt[ip][:, t, m * 128:(m + 1) * 128],
                                    x_items[:, t, n0:n0 + nsz],
                                    start=(t == 0 and ip == 0),
                                    stop=(t == CT - 1 and ip == NPASS_Q - 1))
                        nc.scalar.activation(
                            q_all[:, m, n0:n0 + nsz], pq[:, :nsz],
                            mybir.ActivationFunctionType.Identity,
                            bias=bq_sb[:, m:m + 1])

            # ---------- Phase B: per-sample attention ----------
            with (
                tc.tile_pool(name="xc", bufs=2) as xc_pool,
                tc.tile_pool(name="ksb", bufs=2) as k_pool,
                tc.tile_pool(name="vtsb", bufs=5) as vt_pool,
                tc.tile_pool(name="esb", bufs=3) as e_pool,
                tc.tile_pool(name="etsb", bufs=5) as et_pool,
                tc.tile_pool(name="ytsb", bufs=2) as yt_pool,
                tc.tile_pool(name="zsb", bufs=2) as z_pool,
                tc.tile_pool(name="pacc", bufs=3, space="PSUM") as pacc,
                tc.tile_pool(name="pS", bufs=1, space="PSUM") as pS_pool,
                tc.tile_pool(name="pT", bufs=2, space="PSUM") as pT_pool,
                tc.tile_pool(name="pY", bufs=2, space="PSUM") as pY_pool,
            ):
                for s in range(BPC):
                    py = [pY_pool.tile([128, CI], F32, tag="py", name=f"py{s}_{qi}") for qi, _ in enumerate(QT)]
                    zs = [z_pool.tile([128, len(CHUNKS)], F32, tag="z", name=f"zs{s}_{qi}") for qi, _ in enumerate(QT)]
                    n_kt_total = sum(len(_subtiles(csz)) for _, csz in CHUNKS)
                    kt_idx = 0
                    for ci, (c0, csz) in enumerate(CHUNKS):
                        x_c = xc_pool.tile([128, CT, 512], F32R, tag="xc")
                        nc.sync.dma_start(
                            x_c[:, :, :csz],
                            x_d[s, :, c0:c0 + csz]
                            .rearrange("(t p) n -> p t n", p=128).bitcast(F32R))

                        # K projection: k_sb (Ci-part, Ci-tile, key-col)
                        k_sb = k_pool.tile([128, MI, 512], F32R, tag="ksb")
                        for m in range(MI):
                            pk = pacc.tile([128, 512], F32, tag="acc")
                            for t in range(CT):
                                for ip in range(NPASS_K):
                                    nc.tensor.matmul(
                                        pk[:, :csz],
                                        wkt[ip][:, t, m * 128:(m + 1) * 128],
                                        x_c[:, t, :csz],
                                        start=(t == 0 and ip == 0),
                                        stop=(t == CT - 1 and ip == NPASS_K - 1))
                            nc.scalar.activation(
                                k_sb[:, m, :csz], pk[:, :csz],
                                mybir.ActivationFunctionType.Identity,
                                bias=bk_sb[:, m:m + 1])

                        # V projection -> v^T (key-part, Ci free), x stationary
                        vts = []
                        for (ko, ksz) in _subtiles(csz):
                            pv = pacc.tile([128, CI], F32, tag="acc")
                            np_v = NPASS_V
                            for t in range(CT):
                                for ip in range(np_v):
                                    nc.tensor.matmul(
                                        pv[:ksz, :],
                                        x_c[:, t, ko:ko + ksz],
                                        wvt[ip][:, t, :],
                                        start=(t == 0 and ip == 0),
                                        stop=(t == CT - 1 and ip == np_v - 1))
                            vt = vt_pool.tile([128, CI], F32R, tag="vt")
                            nc.vector.tensor_copy(vt[:ksz, :], pv[:ksz, :])
                            vts.append(vt)

                        # Scores + exp + transpose + AV
                        ets = [et_pool.tile([128, HW], F32R, tag="et", name=f"et{s}_{ci}_{kj}")
                               for kj, _ in enumerate(_subtiles(csz))]
                        for qi, (qo, qsz) in enumerate(QT):
                            ps = pS_pool.tile([128, 512], F32, tag="ps")
                            for m in range(MI):
                                nc.tensor.matmul(
                                    ps[:qsz, :csz],
                                    q_all[:, m, s * HW + qo: s * HW + qo + qsz],
                                    k_sb[:, m, :csz],
                                    start=(m == 0), stop=(m == MI - 1))
                            e_sb = e_pool.tile([128, 512], F32, tag="e")
                            nc.scalar.activation(
                                e_sb[:qsz, :csz], ps[:qsz, :csz],
                                mybir.ActivationFunctionType.Exp,
                                accum_out=zs[qi][:qsz, ci:ci + 1])
                            for kj, (ko, ksz) in enumerate(_subtiles(csz)):
                                pe = pT_pool.tile([128, 128], F32, tag="pt")
                                nc.tensor.transpose(
                                    pe[:ksz, :qsz], e_sb[:qsz, ko:ko + ksz],
                                    ident[:qsz, :qsz])
                                nc.vector.tensor_copy(
                                    ets[kj][:ksz, qo:qo + qsz], pe[:ksz, :qsz])

                        for kj, (ko, ksz) in enumerate(_subtiles(csz)):
                            for qi, (qo, qsz) in enumerate(QT):
                                nc.tensor.matmul(
                                    py[qi][:qsz, :],
                                    ets[kj][:ksz, qo:qo + qsz],
                                    vts[kj][:ksz, :],
                                    start=(kt_idx == 0),
                                    stop=(kt_idx == n_kt_total - 1))
                            kt_idx += 1

                    # normalize + transpose y^T -> y_all (+bv)
                    for qi, (qo, qsz) in enumerate(QT):
                        zsum = z_pool.tile([128, 1], F32, tag="zsum")
                        nc.vector.tensor_reduce(
                            zsum[:qsz, :], zs[qi][:qsz, :],
                            axis=mybir.AxisListType.X, op=mybir.AluOpType.add)
                        rz = z_pool.tile([128, 1], F32, tag="rz")
                        nc.vector.reciprocal(rz[:qsz, :], zsum[:qsz, :])
                        yt = yt_pool.tile([128, CI], F32, tag="yt")
                        nc.scalar.activation(
                            yt[:qsz, :], py[qi][:qsz, :],
                            mybir.ActivationFunctionType.Copy,
                            scale=rz[:qsz, :])
                        for m in range(MI):
                            pyt = pT_pool.tile([128, 128], F32, tag="pt")
                            nc.tensor.transpose(
                                pyt[:, :qsz], yt[:qsz, m * 128:(m + 1) * 128],
                                ident[:qsz, :qsz])
                            nc.scalar.activation(
                                y_all[:, m, s * HW + qo: s * HW + qo + qsz],
                                pyt[:, :qsz],
                                mybir.ActivationFunctionType.Identity,
                                bias=bv_sb[:, m:m + 1])

            # ---------- Phase C: batched output projection ----------
            with (
                tc.tile_pool(name="outsb", bufs=2) as out_pool,
                tc.tile_pool(name="psC", bufs=2, space="PSUM") as psC,
            ):
                for mo in range(CT):
                    out_t = out_pool.tile([128, SAMP], F32, tag="out")
                    for (n0, nsz) in NCH:
                        po = psC.tile([128, 512], F32, tag="po")
                        for m in range(MI):
                            for ip in range(NPASS_O):
                                nc.tensor.matmul(
                                    po[:, :nsz],
                                    wot[ip][:, m, mo * 128:(mo + 1) * 128],
                                    y_all[:, m, n0:n0 + nsz],
                                    start=(m == 0 and ip == 0),
                                    stop=(m == MI - 1 and ip == NPASS_O - 1))
                        nc.scalar.activation(
                            out_t[:, n0:n0 + nsz], po[:, :nsz],
                            mybir.ActivationFunctionType.Identity,
                            bias=bo_sb[:, mo:mo + 1])
                    for s in range(BPC):
                        nc.sync.dma_start(
                            out_d[s, mo * 128:(mo + 1) * 128, :],
                            out_t[:, s * HW:(s + 1) * HW])

    nc.compile()
    return nc


def _tf32_round(x):
    xi = x.view(np.uint32)
    r = ((xi.astype(np.uint64) + 0x1000) & 0xFFFFE000).astype(np.uint32)
    return r.view(np.float32)


def _split_passes(wt, npass):
    """wt: already-transposed weight (contraction-major). Returns npass arrays."""
    wt = np.ascontiguousarray(wt, dtype=np.float32)
    if npass == 1:
        return [wt]
    hi = _tf32_round(wt)
    lo = (wt - hi).astype(np.float32)
    return [hi, lo]


_NC_CACHE = {}


def _get_nc():
    key = (NPASS_Q, NPASS_K, NPASS_V, NPASS_O)
    if key not in _NC_CACHE:
        _NC_CACHE[key] = build_kernel()
    return _NC_CACHE[key]


def kernel(x_window, Wq, bq, Wk, bk, Wv, bv, Wo, bo):
    nc = _get_nc()

    x_window = np.ascontiguousarray(x_window, dtype=np.float32)
    x_flat = x_window.reshape(B, C, THW)

    wqt = _split_passes(np.asarray(Wq, np.float32).T, NPASS_Q)   # (C, CI)
    wkt = _split_passes(np.asarray(Wk, np.float32).T, NPASS_K)
    wvt = _split_passes(np.asarray(Wv, np.float32).T, NPASS_V)
    wot = _split_passes(np.asarray(Wo, np.float32).T, NPASS_O)   # (CI, C)

    shared = {}
    for i, w in enumerate(wqt):
        shared[f"wqt{i}"] = w
    for i, w in enumerate(wkt):
        shared[f"wkt{i}"] = w
    for i, w in enumerate(wvt):
        shared[f"wvt{i}"] = w
    for i, w in enumerate(wot):
        shared[f"wot{i}"] = w
    shared["bq"] = np.ascontiguousarray(np.asarray(bq, np.float32).reshape(MI, 128))
    shared["bk"] = np.ascontiguousarray(np.asarray(bk, np.float32).reshape(MI, 128))
    shared["bv"] = np.ascontiguousarray(np.asarray(bv, np.float32).reshape(MI, 128))
    shared["bo"] = np.ascontiguousarray(np.asarray(bo, np.float32).reshape(CT, 128))

    in_maps = []
    for i in range(NCORES):
        m = dict(shared)
        m["x"] = np.ascontiguousarray(x_flat[i * BPC:(i + 1) * BPC])
        in_maps.append(m)

    res = run_bass_kernel_spmd(nc, in_maps, list(range(NCORES)))
    out = np.concatenate([res.results[i]["out"] for i in range(NCORES)], axis=0)
    return out.reshape(B, C, 1, H, W)
